# revision 2
# baseline (speedup 1.0000x reference)
"""Trainium2 Bass kernel for i1e(z) (exponentially scaled modified Bessel I1).

Input: z float32 (32, 1024, 1024), values in [0.1, 10.1] (positive).
Output: i1e(z), same shape/dtype. Harness gate: rel_err < 2e-2.

Strategy (per core, trivially data-parallel over the leading batch axis):
  - Each of 8 cores gets 4 batches = 4Mi elements, viewed as [128, 32768] f32.
  - Branch-free log-domain approximation:
        i1e(x) = exp(q(u)),  u = ln x,  q = degree-6 minimax fit of
        ln i1e(e^u) on u in [ln 0.1, ln 10.1]  (max |q-h| = 2.05e-3).
  - ScalarE (ACT, natural_log_exp_and_others set) does 3 ops/tile:
        u = Ln(x)                     f32 -> fp16
        a = Square(alpha*u + beta)    = qt6*u^2 + qt5*u + kappa   (qt = -q)
        out = Exp(-acc + q0)          fp16 -> f32
  - VectorE (DVE) does 4 in-place fp16 Horner steps at 2x perf mode:
        acc = (acc + c)*u   for c in [qt4-kappa, qt3, qt2, qt1]
    so  acc = qt(u) - qt0  and  exp(-acc + q0) = exp(q(u)).
  - fp16 intermediates keep DVE at 2x; measured pipeline error vs the
    reference is ~3.5e-3 max / ~1.4e-3 norm (fp16 rounding + fit).
  - Per [128,4096] tile: ACT 3 ops ~11.1us, DVE 4 ops ~9.2us, DMA 4MiB
    ~11.5us -> per-core steady state ~ max(engine) * 8 tiles ~ 92us,
    which sits at the 32MiB/core f32 I/O roofline (~358 GB/s).
"""

import math

import numpy as np

import concourse.bass as bass
import concourse.tile as tile
from concourse import mybir
from concourse.bass_utils import run_bass_kernel_spmd

AF = mybir.ActivationFunctionType
ALU = mybir.AluOpType
F32 = mybir.dt.float32
F16 = mybir.dt.float16

N_CORES = 8
P = 128              # SBUF partitions
FD_TOTAL = 32768     # free-dim elements per partition per core (4Mi total)
TILE_FD = 4096       # free-dim per tile
N_TILES = FD_TOTAL // TILE_FD

# Degree-6 minimax fit (Remez) of h(u) = ln(i1e(e^u)) on [ln 0.0999, ln 10.1001],
# coefficients in increasing power; max |q - h| = 2.05e-3.
Q = [-1.5709213596396698, 0.23323400359402507, -0.2677126883882778,
     -0.010680911716387994, 0.019134093309779863, 0.0013534962789403444,
     -0.0010050521572013385]

# Horner runs on the NEGATED polynomial (qt = -q) so the head Square's
# leading coefficient qt6 is positive; the final Exp uses scale=-1 to flip
# back, and q0 rides in its bias: exp(-acc + q0) = exp(q(u)).
QT = [-v for v in Q]
ALPHA = math.sqrt(QT[6])
BETA = QT[5] / (2.0 * ALPHA)
KAPPA = BETA * BETA
HORNER_C = [QT[4] - KAPPA, QT[3], QT[2], QT[1]]
EXP_BIAS = Q[0]

ACT_BIAS_CONSTS = [BETA, EXP_BIAS]

_CACHED_NC = None


def build_nc(reps: int = 1):
    nc = bass.Bass(trn_type="TRN2")
    x_ext = nc.declare_dram_parameter("x", [P, FD_TOTAL], F32, isOutput=False)
    o_ext = nc.declare_dram_parameter("o", [P, FD_TOTAL], F32, isOutput=True)

    # Register activation-bias constants as const APs, mirroring
    # Bass.__init__'s register_const_ap for 0.0/1.0.
    for i, v in enumerate(ACT_BIAS_CONSTS):
        tns = nc.alloc_sbuf_tensor(f"const-f32-bias{i}", [P, 1], F32)
        nc.gpsimd.memset(tns.ap(), v)
        nc.const_aps.aps[(F32, v)] = tns.ap()
    nc.all_engine_barrier()

    with tile.TileContext(nc) as tc:
        with (
            tc.tile_pool(name="io", bufs=3) as io,
            tc.tile_pool(name="tmp", bufs=2) as tmp,
        ):
            for i in range(N_TILES * reps):
                i = i % N_TILES
                sl = bass.ts(i, TILE_FD)

                x = io.tile([P, TILE_FD], F32, tag="x")
                nc.sync.dma_start(x[:], x_ext[:, sl])

                # ScalarE: u = ln(x), then completed-square Horner head
                # a = qt6*u^2 + qt5*u + kappa.
                u = tmp.tile([P, TILE_FD], F16, tag="u")
                nc.scalar.activation(u[:], x[:], AF.Ln)
                a = tmp.tile([P, TILE_FD], F16, tag="a")
                nc.scalar.activation(a[:], u[:], AF.Square,
                                     scale=ALPHA, bias=BETA)

                # VectorE: in-place fp16 Horner chain (add-then-mult STT).
                for c in HORNER_C:
                    nc.vector.scalar_tensor_tensor(
                        a[:], a[:], float(c), u[:], ALU.add, ALU.mult)

                # ScalarE: i1e = exp(-acc + q0), f32 out.
                out = io.tile([P, TILE_FD], F32, tag="out")
                nc.scalar.activation(out[:], a[:], AF.Exp,
                                     scale=-1.0, bias=EXP_BIAS)

                nc.sync.dma_start(o_ext[:, sl], out[:])

    _split_multi_waits(nc)
    return nc


# TPB compute-instruction ISA formats carry at most ONE sync-wait, but Tile's
# semaphore assignment can attach several (its wait minimality is per-proc,
# not transitive).  Hoist all but one wait onto an InstNoOp inserted right
# before the offending instruction on the same engine.
def _split_multi_waits(nc):
    for bb in nc.main_func.blocks:
        insts = bb.instructions
        i = 0
        while i < len(insts):
            inst = insts[i]
            si = inst.sync_info
            if si is not None and len(si.on_wait) > 1:
                for w in si.on_wait[:-1]:
                    nop = mybir.InstNoOp(
                        name=nc.get_next_instruction_name(),
                        text_hint="wait_split",
                        bass_nofuse=True,
                        engine=inst.engine,
                        sync_info=mybir.SyncInfo(on_wait=[w], on_update=[]),
                    )
                    insts.insert(i, nop)
                    i += 1
                si.on_wait = [si.on_wait[-1]]
            i += 1


def kernel(z: np.ndarray) -> np.ndarray:
    global _CACHED_NC
    assert z.shape == (32, 1024, 1024) and z.dtype == np.float32
    if _CACHED_NC is None:
        _CACHED_NC = build_nc()
    nc = _CACHED_NC

    per_core = 32 // N_CORES
    shards = z.reshape(N_CORES, per_core * 1024 * 1024).reshape(N_CORES, P, FD_TOTAL)
    in_maps = [{"x": np.ascontiguousarray(shards[k])} for k in range(N_CORES)]
    res = run_bass_kernel_spmd(nc, in_maps, list(range(N_CORES))).results
    out = np.concatenate(
        [res[k]["o"].reshape(per_core, 1024, 1024) for k in range(N_CORES)], axis=0
    )
    return out.astype(np.float32)


# revision 3
# speedup vs baseline: 1.3638x; 1.3638x over previous
"""Trainium2 Bass kernel for i1e(z) — v3: fp16 HBM I/O, deg-4 log-domain fit.

Input: z float32 (32, 1024, 1024), values in [0.1, 10.1] (positive).
Output: i1e(z), same shape/dtype. Harness gate: rel_err < 2e-2.

v3 strategy (per core, data-parallel over the leading batch axis):
  - Host casts the f32 input to fp16 before feeding the device and upcasts
    the fp16 device output back to f32: HBM traffic drops from 32MiB to
    16MiB per core (fp16 keeps ~5e-4 rel precision on x and on i1e —
    negligible vs the 2e-2 gate).
  - Branch-free log-domain approximation:
        i1e(x) = exp(q(u)),  u = ln x,  q = degree-4 minimax fit of
        ln i1e(e^u) on [ln 0.0998, ln 10.1005]  (max |q-h| = 8.0e-3).
  - ScalarE (ACT): 2 ops/tile:   u = Ln(x)  [fp16 out],
        out = Exp(-qt4*acc + q0) [fp16 out]  (scale/bias fold the leading
        coefficient and constant term).
  - VectorE (DVE): 3 in-place fp16 STT Horner steps at 2x perf mode on the
    monic negated polynomial:
        acc = (u + qt3/qt4)*u ;  acc = (acc + qt2/qt4)*u ;
        acc = (acc + qt1/qt4)*u       (qt = -q)
    so  -qt4*acc + q0 = q(u) - stray? no: qt4*acc = qt(u)-qt0 => q(u) ok.
  - Simulated end-to-end error (fp16 I/O + fp16 chain): max rel 9.5e-3,
    norm rel 5.7e-3 — 3.5x inside the gate.
  - Per [128,8192] tile: ACT 2 ops ~14.2us, DVE 3 ops ~13.3us, DMA 4MiB
    ~11.5us -> per-core steady state ~4 tiles * 14.2us ~ 57us.
"""

import numpy as np

import concourse.bass as bass
import concourse.tile as tile
from concourse import mybir
from concourse.bass_utils import run_bass_kernel_spmd

AF = mybir.ActivationFunctionType
ALU = mybir.AluOpType
F32 = mybir.dt.float32
F16 = mybir.dt.float16

N_CORES = 8
P = 128              # SBUF partitions
FD_TOTAL = 32768     # free-dim elements per partition per core (4Mi total)
TILE_FD = 8192       # free-dim per tile
N_TILES = FD_TOTAL // TILE_FD

# Degree-4 minimax fit (Remez) of h(u) = ln(i1e(e^u)) on [ln 0.0998, ln 10.1005],
# coefficients in increasing power; max |q - h| = 8.0e-3.
Q = [-1.5759063292958129, 0.22379118317377544, -0.250275080981724,
     -0.0025131655598016806, 0.01070191369933199]

# Horner runs on the monic NEGATED polynomial (qt = -q, divided by qt4):
#   acc = (((u + r3)*u + r2)*u + r1)*u,  r_k = qt_k/qt_4
# then exp(-qt4*acc + q0) = exp(q(u)).
QT = [-v for v in Q]
R3 = QT[3] / QT[4]
R2 = QT[2] / QT[4]
R1 = QT[1] / QT[4]
EXP_SCALE = -QT[4]
EXP_BIAS = Q[0]

ACT_BIAS_CONSTS = [EXP_BIAS]

_CACHED_NC = None


def build_nc(reps: int = 1):
    nc = bass.Bass(trn_type="TRN2")
    x_ext = nc.declare_dram_parameter("x", [P, FD_TOTAL], F16, isOutput=False)
    o_ext = nc.declare_dram_parameter("o", [P, FD_TOTAL], F16, isOutput=True)

    # Register activation-bias constants as const APs, mirroring
    # Bass.__init__'s register_const_ap for 0.0/1.0.
    for i, v in enumerate(ACT_BIAS_CONSTS):
        tns = nc.alloc_sbuf_tensor(f"const-f32-bias{i}", [P, 1], F32)
        nc.gpsimd.memset(tns.ap(), v)
        nc.const_aps.aps[(F32, v)] = tns.ap()
    nc.all_engine_barrier()

    with tile.TileContext(nc) as tc:
        with (
            tc.tile_pool(name="io", bufs=3) as io,
            tc.tile_pool(name="tmp", bufs=2) as tmp,
        ):
            for i in range(N_TILES * reps):
                i = i % N_TILES
                sl = bass.ts(i, TILE_FD)

                x = io.tile([P, TILE_FD], F16, tag="x")
                nc.sync.dma_start(x[:], x_ext[:, sl])

                # ScalarE: u = ln(x), fp16 out.
                u = tmp.tile([P, TILE_FD], F16, tag="u")
                nc.scalar.activation(u[:], x[:], AF.Ln)

                # VectorE: monic Horner chain, first step squares u.
                acc = tmp.tile([P, TILE_FD], F16, tag="acc")
                nc.vector.scalar_tensor_tensor(
                    acc[:], u[:], R3, u[:], ALU.add, ALU.mult)
                nc.vector.scalar_tensor_tensor(
                    acc[:], acc[:], R2, u[:], ALU.add, ALU.mult)
                nc.vector.scalar_tensor_tensor(
                    acc[:], acc[:], R1, u[:], ALU.add, ALU.mult)

                # ScalarE: i1e = exp(-qt4*acc + q0), fp16 out.
                out = io.tile([P, TILE_FD], F16, tag="out")
                nc.scalar.activation(out[:], acc[:], AF.Exp,
                                     scale=EXP_SCALE, bias=EXP_BIAS)

                nc.sync.dma_start(o_ext[:, sl], out[:])

    _split_multi_waits(nc)
    return nc


# TPB compute-instruction ISA formats carry at most ONE sync-wait, but Tile's
# semaphore assignment can attach several (its wait minimality is per-proc,
# not transitive).  Hoist all but one wait onto an InstNoOp inserted right
# before the offending instruction on the same engine.
def _split_multi_waits(nc):
    for bb in nc.main_func.blocks:
        insts = bb.instructions
        i = 0
        while i < len(insts):
            inst = insts[i]
            si = inst.sync_info
            if si is not None and len(si.on_wait) > 1:
                for w in si.on_wait[:-1]:
                    nop = mybir.InstNoOp(
                        name=nc.get_next_instruction_name(),
                        text_hint="wait_split",
                        bass_nofuse=True,
                        engine=inst.engine,
                        sync_info=mybir.SyncInfo(on_wait=[w], on_update=[]),
                    )
                    insts.insert(i, nop)
                    i += 1
                si.on_wait = [si.on_wait[-1]]
            i += 1


def kernel(z: np.ndarray) -> np.ndarray:
    global _CACHED_NC
    assert z.shape == (32, 1024, 1024) and z.dtype == np.float32
    if _CACHED_NC is None:
        _CACHED_NC = build_nc()
    nc = _CACHED_NC

    per_core = 32 // N_CORES
    shards = z.reshape(N_CORES, per_core * 1024 * 1024).reshape(N_CORES, P, FD_TOTAL)
    in_maps = [{"x": shards[k].astype(np.float16)} for k in range(N_CORES)]
    res = run_bass_kernel_spmd(nc, in_maps, list(range(N_CORES))).results
    out = np.concatenate(
        [res[k]["o"].reshape(per_core, 1024, 1024) for k in range(N_CORES)], axis=0
    )
    return out.astype(np.float32)


# revision 10
# speedup vs baseline: 1.4828x; 1.0873x over previous
"""Trainium2 Bass kernel for i1e(z) — v3: fp16 HBM I/O, deg-4 log-domain fit.

Input: z float32 (32, 1024, 1024), values in [0.1, 10.1] (positive).
Output: i1e(z), same shape/dtype. Harness gate: rel_err < 2e-2.

v3 strategy (per core, data-parallel over the leading batch axis):
  - Host casts the f32 input to fp16 before feeding the device and upcasts
    the fp16 device output back to f32: HBM traffic drops from 32MiB to
    16MiB per core (fp16 keeps ~5e-4 rel precision on x and on i1e —
    negligible vs the 2e-2 gate).
  - Branch-free log-domain approximation:
        i1e(x) = exp(q(u)),  u = ln x,  q = degree-4 minimax fit of
        ln i1e(e^u) on [ln 0.0998, ln 10.1005]  (max |q-h| = 8.0e-3).
  - Engine balance (measured sustained costs: ACT ~4.9us/op at fp16 — the
    ScalarE gets 2x accel for 16-bit dtypes; DVE fp16 STT ~5.7-6.6us/op):
    ScalarE takes 3 ops/tile, VectorE only 2.
  - ScalarE (ACT): u = Ln(x) [fp16], completed-square Horner head
        a = Square(alpha*u + beta) = q4*u^2 + q3*u + kappa,
        out = Exp(acc + q0) [fp16].
  - VectorE (DVE): 2 in-place fp16 STT Horner steps at 2x perf mode:
        acc = (a + (q2-kappa))*u ;  acc = (acc + q1)*u
    so acc + q0 = q(u).
  - Loads issue on the SP HWDGE ring (nc.sync), stores on the ACT HWDGE
    ring (nc.scalar): HWDGE DMAs are FIFO per ring, so splitting keeps the
    8MiB of loads and 8MiB of stores per rep flowing in parallel.
  - Simulated end-to-end error (fp16 I/O + fp16 chain): max rel 9.5e-3,
    norm rel 5.7e-3 — 3.5x inside the gate.
  - Per [128,8192] tile: ACT 2 ops ~14.2us, DVE 3 ops ~13.3us, DMA 4MiB
    ~11.5us -> per-core steady state ~4 tiles * 14.2us ~ 57us.
"""

import numpy as np

import concourse.bass as bass
import concourse.tile as tile
from concourse import mybir
from concourse.bass_utils import run_bass_kernel_spmd

AF = mybir.ActivationFunctionType
ALU = mybir.AluOpType
F32 = mybir.dt.float32
F16 = mybir.dt.float16

N_CORES = 8
P = 128              # SBUF partitions
FD_TOTAL = 32768     # free-dim elements per partition per core (4Mi total)
TILE_FD = 8192       # free-dim per tile
N_TILES = FD_TOTAL // TILE_FD

# Degree-4 minimax fit (Remez) of h(u) = ln(i1e(e^u)) on [ln 0.0998, ln 10.1005],
# coefficients in increasing power; max |q - h| = 8.0e-3.
Q = [-1.5759063292958129, 0.22379118317377544, -0.250275080981724,
     -0.0025131655598016806, 0.01070191369933199]

# Completed-square head on ACT: Square(alpha*u + beta) = q4*u^2 + q3*u + kappa
# (q4 > 0), then two STT Horner steps append q2, q1; q0 rides in the Exp bias.
import math as _math
ALPHA = _math.sqrt(Q[4])
BETA = Q[3] / (2.0 * ALPHA)
KAPPA = BETA * BETA
C2 = Q[2] - KAPPA
C1 = Q[1]
EXP_BIAS = Q[0]

ACT_BIAS_CONSTS = [BETA, EXP_BIAS]

_CACHED_NC = None


def build_nc(reps: int = 1):
    nc = bass.Bass(trn_type="TRN2")
    x_ext = nc.declare_dram_parameter("x", [P, FD_TOTAL], F16, isOutput=False)
    o_ext = nc.declare_dram_parameter("o", [P, FD_TOTAL], F16, isOutput=True)

    # Register activation-bias constants as const APs, mirroring
    # Bass.__init__'s register_const_ap for 0.0/1.0.
    for i, v in enumerate(ACT_BIAS_CONSTS):
        tns = nc.alloc_sbuf_tensor(f"const-f32-bias{i}", [P, 1], F32)
        nc.gpsimd.memset(tns.ap(), v)
        nc.const_aps.aps[(F32, v)] = tns.ap()
    nc.all_engine_barrier()

    with tile.TileContext(nc) as tc:
        with (
            tc.tile_pool(name="io", bufs=3) as io,
            tc.tile_pool(name="tmp", bufs=3) as tmp,
        ):
            for i in range(N_TILES * reps):
                i = i % N_TILES
                sl = bass.ts(i, TILE_FD)

                x = io.tile([P, TILE_FD], F16, tag="x")
                nc.sync.dma_start(x[:], x_ext[:, sl])

                # ScalarE: u = ln(x), then the completed-square Horner head.
                u = tmp.tile([P, TILE_FD], F16, tag="u")
                nc.scalar.activation(u[:], x[:], AF.Ln)
                a = tmp.tile([P, TILE_FD], F16, tag="a")
                nc.scalar.activation(a[:], u[:], AF.Square,
                                     scale=ALPHA, bias=BETA)

                # VectorE: two in-place fp16 STT Horner steps.
                nc.vector.scalar_tensor_tensor(
                    a[:], a[:], C2, u[:], ALU.add, ALU.mult)
                nc.vector.scalar_tensor_tensor(
                    a[:], a[:], C1, u[:], ALU.add, ALU.mult)

                # ScalarE: i1e = exp(acc + q0), fp16 out.
                out = io.tile([P, TILE_FD], F16, tag="out")
                nc.scalar.activation(out[:], a[:], AF.Exp,
                                     scale=1.0, bias=EXP_BIAS)

                nc.scalar.dma_start(o_ext[:, sl], out[:])

    _split_multi_waits(nc)
    return nc


# TPB compute-instruction ISA formats carry at most ONE sync-wait, but Tile's
# semaphore assignment can attach several (its wait minimality is per-proc,
# not transitive).  Hoist all but one wait onto an InstNoOp inserted right
# before the offending instruction on the same engine.
def _split_multi_waits(nc):
    for bb in nc.main_func.blocks:
        insts = bb.instructions
        i = 0
        while i < len(insts):
            inst = insts[i]
            si = inst.sync_info
            if si is not None and len(si.on_wait) > 1:
                for w in si.on_wait[:-1]:
                    nop = mybir.InstNoOp(
                        name=nc.get_next_instruction_name(),
                        text_hint="wait_split",
                        bass_nofuse=True,
                        engine=inst.engine,
                        sync_info=mybir.SyncInfo(on_wait=[w], on_update=[]),
                    )
                    insts.insert(i, nop)
                    i += 1
                si.on_wait = [si.on_wait[-1]]
            i += 1


def kernel(z: np.ndarray) -> np.ndarray:
    global _CACHED_NC
    assert z.shape == (32, 1024, 1024) and z.dtype == np.float32
    if _CACHED_NC is None:
        _CACHED_NC = build_nc()
    nc = _CACHED_NC

    per_core = 32 // N_CORES
    shards = z.reshape(N_CORES, per_core * 1024 * 1024).reshape(N_CORES, P, FD_TOTAL)
    in_maps = [{"x": shards[k].astype(np.float16)} for k in range(N_CORES)]
    res = run_bass_kernel_spmd(nc, in_maps, list(range(N_CORES))).results
    out = np.concatenate(
        [res[k]["o"].reshape(per_core, 1024, 1024) for k in range(N_CORES)], axis=0
    )
    return out.astype(np.float32)


# revision 16
# speedup vs baseline: 2.4209x; 1.6326x over previous
"""Trainium2 Bass kernel for i1e(z) — v3: fp16 HBM I/O, deg-4 log-domain fit.

Input: z float32 (32, 1024, 1024), values in [0.1, 10.1] (positive).
Output: i1e(z), same shape/dtype. Harness gate: rel_err < 2e-2.

v3 strategy (per core, data-parallel over the leading batch axis):
  - Host casts the f32 input to fp16 before feeding the device and upcasts
    the fp16 device output back to f32: HBM traffic drops from 32MiB to
    16MiB per core (fp16 keeps ~5e-4 rel precision on x and on i1e —
    negligible vs the 2e-2 gate).
  - Branch-free log-domain approximation:
        i1e(x) = exp(q(u)),  u = ln x,  q = degree-4 minimax fit of
        ln i1e(e^u) on [ln 0.0998, ln 10.1005]  (max |q-h| = 8.0e-3).
  - Engine balance (measured sustained fp16 costs at FD=8192: plain/
    scale-only ACT ops ~4.3-4.9us, ACT ops with a nonzero bias const-AP or
    a scale on Square ~6.1-7.4us, DVE STT ~5.7us effective): ScalarE takes
    3 bias-free ops/tile, VectorE 2 STT.
  - Variable shift kills the cubic term so the head needs no bias:
    u = ln(gamma*x) with gamma = exp(q3/(4*q4)) folded into Ln's scale
    (free immediate) makes P(u) = q4*u^4 + p2*u^2 + p1*u + p0.
  - ScalarE (ACT): u = Ln(gamma*x) [fp16], a = Square(u) [plain],
    out = Exp(q4 * acc) [scale immediate, no bias].
  - VectorE (DVE): 2 in-place fp16 STT Horner steps on the monic poly:
        acc = (a + p2/q4)*u ;  acc = (acc + p1/q4)*u
    so q4*acc = P(u) - p0.
  - The constant exp(p0) is folded into the host-side fp16->f32 upcast of
    the output (a scalar multiply in the same pass).
  - Loads issue on the SP HWDGE ring (nc.sync), stores on the ACT HWDGE
    ring (nc.scalar): HWDGE DMAs are FIFO per ring, so splitting keeps the
    8MiB of loads and 8MiB of stores per rep flowing in parallel.
  - Simulated end-to-end error (fp16 I/O + fp16 chain): max rel 9.5e-3,
    norm rel 5.7e-3 — 3.5x inside the gate.
  - Per [128,8192] tile: ACT 2 ops ~14.2us, DVE 3 ops ~13.3us, DMA 4MiB
    ~11.5us -> per-core steady state ~4 tiles * 14.2us ~ 57us.
"""

import numpy as np

import concourse.bass as bass
import concourse.tile as tile
from concourse import mybir
from concourse.bass_utils import run_bass_kernel_spmd

AF = mybir.ActivationFunctionType
ALU = mybir.AluOpType
F32 = mybir.dt.float32
F16 = mybir.dt.float16

N_CORES = 8
P = 128              # SBUF partitions
FD_TOTAL = 32768     # free-dim elements per partition per core (4Mi total)
TILE_FD = 8192       # free-dim per tile
N_TILES = FD_TOTAL // TILE_FD

# Degree-4 minimax fit (Remez) of h(u) = ln(i1e(e^u)) on [ln 0.0998, ln 10.1005],
# coefficients in increasing power; max |q - h| = 8.0e-3.
Q = [-1.5759063292958129, 0.22379118317377544, -0.250275080981724,
     -0.0025131655598016806, 0.01070191369933199]

# Shift the variable so the cubic term vanishes: with s = q3/(4*q4) and
# u' = u + s = ln(gamma*x), gamma = e^s, the polynomial becomes
#   P(u') = q4*u'^4 + p2*u'^2 + p1*u' + p0   (no cubic, q4 > 0).
# Horner runs on the monic P/q4; q4 rides in Exp's scale immediate and
# exp(p0) is applied on the host during the f32 upcast.
import math as _math
_S = Q[3] / (4.0 * Q[4])
GAMMA = _math.exp(_S)
# P(t) = sum_k Q[k] * (t - _S)^k, expanded in float64 at import time:
_pw = np.array([1.0])
_Pc = np.zeros(5)
for _k in range(5):
    _Pc[: len(_pw)] += Q[_k] * _pw
    _pw = np.convolve(_pw, np.array([-_S, 1.0]))
P0, P1, P2, P3, P4 = [float(v) for v in _Pc]
assert abs(P3) < 1e-15 and P4 > 0
C2 = P2 / P4
C1 = P1 / P4
EXP_SCALE = P4
HOST_SCALE = _math.exp(P0)

_CACHED_NC = None


def build_nc(reps: int = 1):
    nc = bass.Bass(trn_type="TRN2")
    x_ext = nc.declare_dram_parameter("x", [P, FD_TOTAL], F16, isOutput=False)
    o_ext = nc.declare_dram_parameter("o", [P, FD_TOTAL], F16, isOutput=True)

    nc.all_engine_barrier()

    with tile.TileContext(nc) as tc:
        with (
            tc.tile_pool(name="io", bufs=3) as io,
            tc.tile_pool(name="tmp", bufs=3) as tmp,
        ):
            for i in range(N_TILES * reps):
                i = i % N_TILES
                sl = bass.ts(i, TILE_FD)

                x = io.tile([P, TILE_FD], F16, tag="x")
                nc.sync.dma_start(x[:], x_ext[:, sl])

                # ScalarE: u = ln(gamma*x), then the plain-Square Horner head.
                u = tmp.tile([P, TILE_FD], F16, tag="u")
                nc.scalar.activation(u[:], x[:], AF.Ln, scale=GAMMA)
                a = tmp.tile([P, TILE_FD], F16, tag="a")
                nc.scalar.activation(a[:], u[:], AF.Square)

                # VectorE: two in-place fp16 STT Horner steps (monic poly).
                nc.vector.scalar_tensor_tensor(
                    a[:], a[:], C2, u[:], ALU.add, ALU.mult)
                nc.vector.scalar_tensor_tensor(
                    a[:], a[:], C1, u[:], ALU.add, ALU.mult)

                # ScalarE: exp(q4*acc) = i1e / exp(p0), fp16 out.
                out = io.tile([P, TILE_FD], F16, tag="out")
                nc.scalar.activation(out[:], a[:], AF.Exp, scale=EXP_SCALE)

                nc.scalar.dma_start(o_ext[:, sl], out[:])

    _split_multi_waits(nc)
    return nc


# TPB compute-instruction ISA formats carry at most ONE sync-wait, but Tile's
# semaphore assignment can attach several (its wait minimality is per-proc,
# not transitive).  Hoist all but one wait onto an InstNoOp inserted right
# before the offending instruction on the same engine.
def _split_multi_waits(nc):
    for bb in nc.main_func.blocks:
        insts = bb.instructions
        i = 0
        while i < len(insts):
            inst = insts[i]
            si = inst.sync_info
            if si is not None and len(si.on_wait) > 1:
                for w in si.on_wait[:-1]:
                    nop = mybir.InstNoOp(
                        name=nc.get_next_instruction_name(),
                        text_hint="wait_split",
                        bass_nofuse=True,
                        engine=inst.engine,
                        sync_info=mybir.SyncInfo(on_wait=[w], on_update=[]),
                    )
                    insts.insert(i, nop)
                    i += 1
                si.on_wait = [si.on_wait[-1]]
            i += 1


def kernel(z: np.ndarray) -> np.ndarray:
    global _CACHED_NC
    assert z.shape == (32, 1024, 1024) and z.dtype == np.float32
    if _CACHED_NC is None:
        _CACHED_NC = build_nc()
    nc = _CACHED_NC

    per_core = 32 // N_CORES
    shards = z.reshape(N_CORES, per_core * 1024 * 1024).reshape(N_CORES, P, FD_TOTAL)
    in_maps = [{"x": shards[k].astype(np.float16)} for k in range(N_CORES)]
    res = run_bass_kernel_spmd(nc, in_maps, list(range(N_CORES))).results
    out = np.concatenate(
        [res[k]["o"].reshape(per_core, 1024, 1024) for k in range(N_CORES)], axis=0
    )
    # Upcast and apply the folded constant exp(p0) in one host pass.
    return out.astype(np.float32) * np.float32(HOST_SCALE)
